# revision 1
# baseline (speedup 1.0000x reference)
"""Trainium2 Bass kernel: cached causal self-attention (dense transformer block).

Full module: y = CausalAttn(x; Wq, Wk, Wv) @ Wo.T + bo with
  B=4, S=2048, E=2048, H=16 heads, Dh=128, fp32 inputs.

Distribution: 8-way tensor parallel over heads (2 heads per NeuronCore).
Each core computes Q/K/V projections for its 2 heads (contraction over the
full embedding dim), causal-softmax attention for those heads, and a partial
output projection y_c = ctx_c @ Wo[:, c*256:(c+1)*256].T.  The host sums the
8 partials and adds the bias (the cross-head reduction of the output
projection), avoiding on-device collectives.

Matmuls run in float32r (single-pass fp32 on the PE array, ~1e-4 relative
error, 4x the throughput of exact fp32).  Layout choices:
  - x is pre-transposed on the host (xT [E, B*S]) so every contraction over
    E has E on the SBUF partition dim with clean contiguous DMAs.
  - scores are computed transposed (sT[k, q]) so no on-chip transpose of the
    attention matrix is ever needed: exp(sT) feeds the attn@V matmul as the
    moving operand directly (ctxT[d, q] = sum_k V[k, d]^T-free exp(sT)[k, q]).
  - softmax denominators (column sums of exp(sT)) come from a ones-vector
    matmul; they are re-laid-out to [s partitions, 1] via a tiny DMA fold +
    PE transpose so the normalization happens per-partition at the output
    projection eviction (per head, before the two heads' partials mix).
"""

import math

import ml_dtypes
import numpy as np

import concourse.bacc as bacc
import concourse.mybir as mybir
import concourse.tile as tile
from concourse.bass_utils import run_bass_kernel_spmd

F32 = mybir.dt.float32
F32R = mybir.dt.float32r
AF = mybir.ActivationFunctionType
ALU = mybir.AluOpType

NEG = -1.0e30

# Full-problem constants
EMB = 2048
N_HEADS = 16
HEAD_DIM = 128
B_FULL = 4
S_FULL = 2048
N_CORES = 8
HPC = N_HEADS // N_CORES  # heads per core = 2


def build(B=B_FULL, S=S_FULL, E=EMB, hpc=HPC, DH=HEAD_DIM, CH=512, reps=1):
    """Build the per-core Bass program (same program on all 8 cores)."""
    assert hpc == 2, "y eviction chain is written for 2 heads per core"
    SB = B * S
    DHC = hpc * DH          # per-core head dims (256)
    NE = E // 128           # e-tiles (contraction tiles)
    NCH = S // CH           # 512-wide chunks per sequence
    KPC = CH // 128         # k-tiles per chunk (4)
    NST = S // 128          # 128-row s-tiles per sequence
    NOC = E // CH           # output chunks
    scale = 1.0 / math.sqrt(DH)

    nc = bacc.Bacc("TRN2", target_bir_lowering=False, debug=False,
                   num_devices=N_CORES)

    xT = nc.dram_tensor("xT", [E, SB], F32R, kind="ExternalInput")
    wqT = nc.dram_tensor("wqT", [E, DHC], F32R, kind="ExternalInput")
    wkT = nc.dram_tensor("wkT", [E, DHC], F32R, kind="ExternalInput")
    wvT = nc.dram_tensor("wvT", [E, DHC], F32R, kind="ExternalInput")
    woT = nc.dram_tensor("woT", [DHC, E], F32R, kind="ExternalInput")
    masks = nc.dram_tensor("masks", [128, CH], mybir.dt.bfloat16, kind="ExternalInput")
    ones = nc.dram_tensor("ones", [128, 1], F32R, kind="ExternalInput")
    y = nc.dram_tensor("y", [SB, E], F32, kind="ExternalOutput")

    with tile.TileContext(nc) as tc:
        with (
            tc.tile_pool(name="wpool", bufs=1) as wpool,
            tc.tile_pool(name="xtp", bufs=2) as xtp,
            tc.tile_pool(name="qkv", bufs=1) as qkv,
            tc.tile_pool(name="expp", bufs=4) as expp,
            tc.tile_pool(name="denp", bufs=1) as denp_sb,
            tc.tile_pool(name="dramp", bufs=2, space="DRAM") as dramp,
            tc.tile_pool(name="yout", bufs=2) as yout,
            tc.tile_pool(name="ps_mm", bufs=3, space="PSUM") as ps_mm,
            tc.tile_pool(name="ps_proj", bufs=2, space="PSUM") as ps_proj,
            tc.tile_pool(name="ps_av", bufs=2, space="PSUM") as ps_av,
            tc.tile_pool(name="ps_den", bufs=1, space="PSUM") as ps_den,
        ):
            # Resident weights / constants (one batched DMA each)
            wq_sb = wpool.tile([128, NE, DHC], F32R, tag="wq")
            wk_sb = wpool.tile([128, NE, DHC], F32R, tag="wk")
            wv_sb = wpool.tile([128, NE, DHC], F32R, tag="wv")
            wo_sb = wpool.tile([128, hpc, E], F32R, tag="wo")
            xT_r = xT.rearrange("(t p) s -> p t s", p=128)
            NEH = NE // 2
            wq_r = wqT.rearrange("(t p) d -> p t d", p=128)
            wk_r = wkT.rearrange("(t p) d -> p t d", p=128)
            # halves: the first Q/K accumulation consumes e-tiles in order,
            # so the low half arriving first starts the PE sooner
            nc.sync.dma_start(wq_sb[:, 0:NE // 2, :], wq_r[:, 0:NE // 2, :])
            nc.sync.dma_start(wk_sb[:, 0:NE // 2, :], wk_r[:, 0:NE // 2, :])
            nc.sync.dma_start(wq_sb[:, NE // 2:NE, :], wq_r[:, NE // 2:NE, :])
            nc.sync.dma_start(wk_sb[:, NE // 2:NE, :], wk_r[:, NE // 2:NE, :])
            xpre0 = None
            if reps == 1:
                # prefetch the first x chunk ahead of the remaining (not yet
                # needed) weights so the first Q matmuls start ~20us sooner
                x0a = xtp.tile([128, NEH, CH], F32R, tag="xta", name="x0a")
                nc.sync.dma_start(x0a[:], xT_r[:, 0:NEH, 0:CH])
                x0b = xtp.tile([128, NEH, CH], F32R, tag="xtb", name="x0b")
                nc.sync.dma_start(x0b[:], xT_r[:, NEH:NE, 0:CH])
                xpre0 = ((0, 0), x0a, x0b)
            nc.sync.dma_start(wv_sb[:], wvT.rearrange("(t p) d -> p t d", p=128))
            nc.sync.dma_start(wo_sb[:], woT.rearrange("(h p) e -> p h e", p=128))
            mask_sb = wpool.tile([128, CH], mybir.dt.bfloat16, tag="mask")
            nc.sync.dma_start(mask_sb[:], masks[:, :])
            ones_sb = wpool.tile([128, 1], F32R, tag="ones")
            nc.sync.dma_start(ones_sb[:], ones[:, :])

            import contextlib
            rep_cm = tc.For_i(0, reps, 1) if reps > 1 else contextlib.nullcontext()
            with rep_cm:
              pending_proj = None
              for b in range(B):
                  s0 = b * S
                  # ---------------- Phase A: Q/K/V projections -------------
                  qT = qkv.tile([128, hpc, S], F32R, tag="qT")
                  kT = qkv.tile([128, hpc, S], F32R, tag="kT")
                  v_sb = qkv.tile([128, NST, DHC], F32R, tag="v")
                  if b == 0:
                      xpre = xpre0
                  for ch in range(NCH):
                      c0 = ch * CH
                      if xpre is not None and xpre[0] == (b, ch):
                          xta, xtb = xpre[1], xpre[2]
                      else:
                          xta = xtp.tile([128, NEH, CH], F32R, tag="xta")
                          nc.sync.dma_start(xta[:],
                                            xT_r[:, 0:NEH, s0 + c0:s0 + c0 + CH])
                          xtb = xtp.tile([128, NEH, CH], F32R, tag="xtb")
                          nc.sync.dma_start(xtb[:],
                                            xT_r[:, NEH:NE, s0 + c0:s0 + c0 + CH])
                      if ch + 1 < NCH or b + 1 < B:
                          nb_, nch = (b, ch + 1) if ch + 1 < NCH else (b + 1, 0)
                          n0 = nb_ * S + nch * CH
                          xna = xtp.tile([128, NEH, CH], F32R, tag="xta",
                                         name="xna")
                          nc.sync.dma_start(xna[:], xT_r[:, 0:NEH, n0:n0 + CH])
                          xnb = xtp.tile([128, NEH, CH], F32R, tag="xtb",
                                         name="xnb")
                          nc.sync.dma_start(xnb[:], xT_r[:, NEH:NE, n0:n0 + CH])
                          xpre = ((nb_, nch), xna, xnb)
                      else:
                          xpre = None

                      def xslice(et, lo=None, hi=None):
                          t = xta if et < NEH else xtb
                          e = et if et < NEH else et - NEH
                          if lo is None:
                              return t[:, e, :]
                          return t[:, e, lo:hi]

                      for h in range(hpc):
                          qp = ps_mm.tile([128, CH], F32, tag="qkvp")
                          for et in range(NE):
                              nc.tensor.matmul(
                                  qp[:], wq_sb[:, et, h * DH:(h + 1) * DH],
                                  xslice(et),
                                  start=(et == 0), stop=(et == NE - 1))
                          nc.scalar.activation(qT[:, h, c0:c0 + CH], qp[:],
                                               AF.Identity, scale=scale)
                          kp = ps_mm.tile([128, CH], F32, tag="qkvp")
                          for et in range(NE):
                              nc.tensor.matmul(
                                  kp[:], wk_sb[:, et, h * DH:(h + 1) * DH],
                                  xslice(et),
                                  start=(et == 0), stop=(et == NE - 1))
                          nc.scalar.activation(kT[:, h, c0:c0 + CH], kp[:], AF.Identity)
                      for st in range(KPC):
                          vp = ps_mm.tile([128, DHC], F32, tag="qkvp")
                          for et in range(NE):
                              nc.tensor.matmul(
                                  vp[:], xslice(et, st * 128, (st + 1) * 128),
                                  wv_sb[:, et, :],
                                  start=(et == 0), stop=(et == NE - 1))
                          nc.scalar.activation(v_sb[:, ch * KPC + st, :], vp[:],
                                               AF.Identity)

                  if pending_proj is not None:
                      emit_proj(*pending_proj)
                      pending_proj = None

                  # ------- Phase B+C: attention with interleaved projection ----
                  # Per 512-chunk g: both heads' attention for queries in g,
                  # per-chunk softmax denominators, then the output projection
                  # for chunk g's s-tiles.  This spreads the y write-out DMA
                  # into the attention window (which has no DMA traffic of its
                  # own) instead of bunching it at the end of the batch.
                  ctxT = qkv.tile([128, hpc, S], F32R, tag="ctxT")
                  rdenT = [denp_sb.tile([128, NST], F32, tag=f"rden{h}",
                                        name=f"rden{h}")
                           for h in range(hpc)]
                  def emit_proj(pctx, prden, ps0, g):
                      for st in range(g * KPC, (g + 1) * KPC):
                          for oc in range(NOC):
                              o0 = oc * CH
                              p0 = ps_proj.tile([128, CH], F32, tag="proj")
                              nc.tensor.matmul(
                                  p0[:], pctx[:, 0, st * 128:(st + 1) * 128],
                                  wo_sb[:, 0, o0:o0 + CH], start=True, stop=True)
                              p1 = ps_proj.tile([128, CH], F32, tag="proj")
                              nc.tensor.matmul(
                                  p1[:], pctx[:, 1, st * 128:(st + 1) * 128],
                                  wo_sb[:, 1, o0:o0 + CH], start=True, stop=True)
                              ysb = yout.tile([128, CH], F32, tag="ysb")
                              if (st + oc) % 2 == 0:
                                  nc.scalar.activation(
                                      ysb[:], p0[:], AF.Identity,
                                      scale=prden[0][:, st:st + 1])
                              else:
                                  nc.vector.tensor_scalar(
                                      ysb[:], p0[:], prden[0][:, st:st + 1],
                                      None, op0=ALU.mult)
                              nc.vector.scalar_tensor_tensor(
                                  ysb[:], p1[:], prden[1][:, st:st + 1], ysb[:],
                                  op0=ALU.mult, op1=ALU.add)
                              nc.gpsimd.dma_start(
                                  y[ps0 + st * 128:ps0 + (st + 1) * 128,
                                    o0:o0 + CH], ysb[:])

                  for g in range(NCH):
                      for h in range(hpc):
                          nk = KPC * (g + 1)
                          avp = ps_av.tile([128, CH], F32, tag="av")
                          dnp = ps_den.tile([1, CH], F32, tag="den")
                          for kt in range(nk):
                              # diagonal k-tiles: queries q < 128j are fully
                              # masked -- compute only the suffix [off, CH)
                              j = kt - (nk - KPC)
                              off = 128 * j if j > 0 else 0
                              w = CH - off
                              sp = ps_mm.tile([128, CH], F32, tag="qkvp", name="sp")
                              nc.tensor.matmul(
                                  sp[:, off:], kT[:, h, kt * 128:(kt + 1) * 128],
                                  qT[:, h, g * CH + off:(g + 1) * CH],
                                  start=True, stop=True)
                              if j >= 0:
                                  # mask col c: masked iff c < p (strict tri)
                                  nc.vector.tensor_add(sp[:, off:], sp[:, off:],
                                                       mask_sb[:, 0:w])
                              ex = expp.tile([128, CH], F32R, tag="ex")
                              nc.scalar.activation(ex[:, off:], sp[:, off:], AF.Exp)
                              nc.tensor.matmul(
                                  avp[:, off:], v_sb[:, kt, h * DH:(h + 1) * DH],
                                  ex[:, off:],
                                  start=(kt == 0), stop=(kt == nk - 1),
                                  skip_group_check=True)
                              nc.tensor.matmul(
                                  dnp[:, off:], ones_sb[:], ex[:, off:],
                                  start=(kt == 0), stop=(kt == nk - 1),
                                  skip_group_check=True)
                          nc.scalar.activation(ctxT[:, h, g * CH:(g + 1) * CH],
                                               avp[:], AF.Identity)
                          den_ch = denp_sb.tile([1, CH], F32, tag="den_ch")
                          nc.scalar.activation(den_ch[:], dnp[:], AF.Identity)
                          # bounce the 2KB denominator row through DRAM to
                          # transpose it to [128 s-partitions, KPC] with pure
                          # address-stream APs, all on the idle Pool engine --
                          # the PE never sits in the denominator chain
                          den_d = dramp.tile([1, CH], F32, tag="den_d")
                          nc.gpsimd.dma_start(den_d[:], den_ch[:])
                          den_t = denp_sb.tile([128, KPC], F32, tag="den_t")
                          nc.gpsimd.dma_start(
                              den_t[:],
                              den_d[:].rearrange("p (j q) -> (p q) j", j=KPC))
                          nc.vector.reciprocal(
                              rdenT[h][:, g * KPC:(g + 1) * KPC], den_t[:])
                      if g > 0:
                          emit_proj(ctxT, rdenT, s0, g - 1)
                  pending_proj = (ctxT, rdenT, s0, NCH - 1)
              if pending_proj is not None:
                  emit_proj(*pending_proj)
                  pending_proj = None
    nc.finalize()
    return nc


def host_consts(S=S_FULL, CH=512):
    """Mask / ones / identity constant inputs."""
    KPC = CH // 128
    NST = S // 128
    p = np.arange(128)[:, None]
    c = np.arange(CH)[None, :]
    # strict lower triangle: masked iff c < p (diagonal k-tile suffix mask)
    masks = np.where(c < p, np.float32(NEG), np.float32(0.0))
    masks = np.ascontiguousarray(masks.astype(ml_dtypes.bfloat16))
    return {
        "masks": masks,
        "ones": np.ones((128, 1), dtype=np.float32),
    }


def host_inputs(x, Wq, Wk, Wv, Wo, B=B_FULL, S=S_FULL, E=EMB, hpc=HPC,
                DH=HEAD_DIM, CH=512):
    """Shard + lay out the full inputs for the 8 cores."""
    SB = B * S
    DHC = hpc * DH
    xT = np.ascontiguousarray(x.reshape(SB, E).T)
    consts = host_consts(S, CH)

    in_maps = []
    for c in range(N_CORES):
        lo, hi = c * DHC, (c + 1) * DHC
        in_maps.append({
            "xT": xT,
            "wqT": np.ascontiguousarray(Wq[lo:hi, :].T),
            "wkT": np.ascontiguousarray(Wk[lo:hi, :].T),
            "wvT": np.ascontiguousarray(Wv[lo:hi, :].T),
            "woT": np.ascontiguousarray(Wo[:, lo:hi].T),
            **consts,
        })
    return in_maps


def kernel(x, Wq, Wk, Wv, Wo, bo):
    x = np.asarray(x, dtype=np.float32)
    Wq = np.asarray(Wq, dtype=np.float32)
    Wk = np.asarray(Wk, dtype=np.float32)
    Wv = np.asarray(Wv, dtype=np.float32)
    Wo = np.asarray(Wo, dtype=np.float32)
    bo = np.asarray(bo, dtype=np.float32)

    nc = build()
    in_maps = host_inputs(x, Wq, Wk, Wv, Wo)
    res = run_bass_kernel_spmd(nc, in_maps, list(range(N_CORES)))
    y = res.results[0]["y"].astype(np.float64)
    for c in range(1, N_CORES):
        y += res.results[c]["y"]
    y = (y + bo).astype(np.float32)
    return y.reshape(B_FULL, S_FULL, EMB)



# revision 2
# speedup vs baseline: 1.2808x; 1.2808x over previous
"""Trainium2 Bass kernel: cached causal self-attention (dense transformer block).

Full module: y = CausalAttn(x; Wq, Wk, Wv) @ Wo.T + bo with
  B=4, S=2048, E=2048, H=16 heads, Dh=128, fp32 inputs.

Distribution: 8-way tensor parallel over heads (2 heads per NeuronCore).
Each core computes Q/K/V projections for its 2 heads, causal-softmax
attention for those heads, and a partial output projection
y_c = ctx_c @ Wo[:, c*256:(c+1)*256].T.  The host sums the 8 partials and
adds the bias, avoiding on-device collectives.

v2 changes vs the fp32r baseline:
  - all matmul operands in bf16 (PSUM accumulation stays fp32): halves
    SBUF/HBM traffic and PE power (the fp32r version hit sustained power
    throttling, ~1.8 GHz effective vs 2.4 GHz peak).
  - attention inner loop software-pipelined: the scores matmul for k-tile
    t+1 issues before attn@V of tile t, so the Exp activation latency hides
    under PE work instead of stalling the accumulation chain.
  - output-projection matmuls run as a job queue drained one tile per
    k-tile step (and between QKV chains), filling the remaining PE bubbles
    of the attention dependency chain.
  - softmax normalization moved off the y path: 1/denominator is broadcast
    across partitions once per (chunk, head) and applied at the ctx PSUM
    eviction, so both heads' projection partials accumulate in one PSUM
    bank and evict with a single copy (the baseline spent 2-3 Vector ops
    per y tile on it).  The denominator transpose DRAM bounce is gone.
  - y partials transfer in bf16 (host sums in fp32).
"""

import math

import ml_dtypes
import numpy as np

import concourse.bacc as bacc
import concourse.mybir as mybir
import concourse.tile as tile
from concourse.bass_utils import run_bass_kernel_spmd

F32 = mybir.dt.float32
BF16 = mybir.dt.bfloat16
AF = mybir.ActivationFunctionType
ALU = mybir.AluOpType

NEG = -1.0e30

# Full-problem constants
EMB = 2048
N_HEADS = 16
HEAD_DIM = 128
B_FULL = 4
S_FULL = 2048
N_CORES = 8
HPC = N_HEADS // N_CORES  # heads per core = 2


def build(B=B_FULL, S=S_FULL, E=EMB, hpc=HPC, DH=HEAD_DIM, CH=512):
    """Build the per-core Bass program (same program on all 8 cores)."""
    assert hpc == 2, "projection accumulation is written for 2 heads per core"
    SB = B * S
    DHC = hpc * DH          # per-core head dims (256)
    NE = E // 128           # e-tiles (contraction tiles)
    NEH = NE // 2
    NCH = S // CH           # 512-wide chunks per sequence
    KPC = CH // 128         # k-tiles per chunk (4)
    NST = S // 128          # 128-row s-tiles per sequence
    NOC = E // CH           # output chunks
    scale = 1.0 / math.sqrt(DH)

    nc = bacc.Bacc("TRN2", target_bir_lowering=False, debug=False,
                   num_devices=N_CORES)

    xT = nc.dram_tensor("xT", [E, SB], BF16, kind="ExternalInput")
    wqT = nc.dram_tensor("wqT", [E, DHC], BF16, kind="ExternalInput")
    wkT = nc.dram_tensor("wkT", [E, DHC], BF16, kind="ExternalInput")
    wvT = nc.dram_tensor("wvT", [E, DHC], BF16, kind="ExternalInput")
    woT = nc.dram_tensor("woT", [DHC, E], BF16, kind="ExternalInput")
    masks = nc.dram_tensor("masks", [128, CH], BF16, kind="ExternalInput")
    ones = nc.dram_tensor("ones", [128, 1], BF16, kind="ExternalInput")
    y = nc.dram_tensor("y", [SB, E], BF16, kind="ExternalOutput")

    with tile.TileContext(nc) as tc:
        with (
            tc.tile_pool(name="wpool", bufs=1) as wpool,
            tc.tile_pool(name="xtp", bufs=2) as xtp,
            tc.tile_pool(name="qkv", bufs=1) as qkv,
            tc.tile_pool(name="expp", bufs=4) as expp,
            tc.tile_pool(name="denp", bufs=2) as denp_sb,
            tc.tile_pool(name="yout", bufs=3) as yout,
            tc.tile_pool(name="ps_mm", bufs=3, space="PSUM") as ps_mm,
            tc.tile_pool(name="ps_proj", bufs=2, space="PSUM") as ps_proj,
            tc.tile_pool(name="ps_av", bufs=2, space="PSUM") as ps_av,
            tc.tile_pool(name="ps_den", bufs=1, space="PSUM") as ps_den,
        ):
            # Resident weights / constants.  The first Q chain consumes
            # e-tiles in order, so stage the DMAs so its head-of-stream
            # tiles (wq + x quarter 0) land first.
            wq_sb = wpool.tile([128, NE, DHC], BF16, tag="wq")
            wk_sb = wpool.tile([128, NE, DHC], BF16, tag="wk")
            wv_sb = wpool.tile([128, NE, DHC], BF16, tag="wv")
            wo_sb = wpool.tile([128, hpc, E], BF16, tag="wo")
            xT_r = xT.rearrange("(t p) s -> p t s", p=128)
            wq_r = wqT.rearrange("(t p) d -> p t d", p=128)
            wk_r = wkT.rearrange("(t p) d -> p t d", p=128)

            nc.sync.dma_start(wq_sb[:, 0:4, :], wq_r[:, 0:4, :])
            x0a = xtp.tile([128, NEH, CH], BF16, tag="xta", name="x0a")
            x0b = xtp.tile([128, NEH, CH], BF16, tag="xtb", name="x0b")
            nc.sync.dma_start(x0a[:, 0:4, :], xT_r[:, 0:4, 0:CH])
            nc.sync.dma_start(wq_sb[:, 4:NEH, :], wq_r[:, 4:NEH, :])
            nc.sync.dma_start(x0a[:, 4:NEH, :], xT_r[:, 4:NEH, 0:CH])
            nc.sync.dma_start(wq_sb[:, NEH:NE, :], wq_r[:, NEH:NE, :])
            nc.sync.dma_start(x0b[:], xT_r[:, NEH:NE, 0:CH])
            nc.sync.dma_start(wk_sb[:, 0:NEH, :], wk_r[:, 0:NEH, :])
            nc.sync.dma_start(wk_sb[:, NEH:NE, :], wk_r[:, NEH:NE, :])
            xpre0 = ((0, 0), x0a, x0b)
            nc.sync.dma_start(wv_sb[:], wvT.rearrange("(t p) d -> p t d", p=128))
            nc.sync.dma_start(wo_sb[:], woT.rearrange("(h p) e -> p h e", p=128))
            mask_sb = wpool.tile([128, CH], BF16, tag="mask")
            nc.sync.dma_start(mask_sb[:], masks[:, :])
            ones_sb = wpool.tile([128, 1], BF16, tag="ones")
            nc.sync.dma_start(ones_sb[:], ones[:, :])

            # ---- output-projection job queue --------------------------
            # A job is one y tile (st, oc): both heads' partials accumulate
            # into one PSUM tile, single-op eviction (alternating engine),
            # DMA from the Sync engine (Pool stays light).
            proj_jobs = []
            evict_flip = [0]

            def emit_proj_job(job):
                ctxn, s0, st, oc = job
                o0 = oc * CH
                p = ps_proj.tile([128, CH], F32, tag="proj")
                nc.tensor.matmul(p[:], ctxn[:, 0, st * 128:(st + 1) * 128],
                                 wo_sb[:, 0, o0:o0 + CH],
                                 start=True, stop=False)
                nc.tensor.matmul(p[:], ctxn[:, 1, st * 128:(st + 1) * 128],
                                 wo_sb[:, 1, o0:o0 + CH],
                                 start=False, stop=True)
                ysb = yout.tile([128, CH], BF16, tag="ysb")
                if evict_flip[0] == 0:
                    nc.scalar.copy(ysb[:], p[:])
                else:
                    nc.vector.tensor_copy(ysb[:], p[:])
                evict_flip[0] ^= 1
                nc.sync.dma_start(
                    y[s0 + st * 128:s0 + (st + 1) * 128, o0:o0 + CH], ysb[:])

            def pop_proj(n=1):
                for _ in range(min(n, len(proj_jobs))):
                    emit_proj_job(proj_jobs.pop(0))

            xpre = xpre0
            for b in range(B):
                s0 = b * S
                # ---------------- Phase A: Q/K/V projections -------------
                qT = qkv.tile([128, hpc, S], BF16, tag="qT")
                kT = qkv.tile([128, hpc, S], BF16, tag="kT")
                v_sb = qkv.tile([128, NST, DHC], BF16, tag="v")
                for ch in range(NCH):
                    c0 = ch * CH
                    if xpre is not None and xpre[0] == (b, ch):
                        xta, xtb = xpre[1], xpre[2]
                    else:
                        xta = xtp.tile([128, NEH, CH], BF16, tag="xta")
                        nc.sync.dma_start(xta[:],
                                          xT_r[:, 0:NEH, s0 + c0:s0 + c0 + CH])
                        xtb = xtp.tile([128, NEH, CH], BF16, tag="xtb")
                        nc.sync.dma_start(xtb[:],
                                          xT_r[:, NEH:NE, s0 + c0:s0 + c0 + CH])
                    if ch + 1 < NCH or b + 1 < B:
                        nb_, nch = (b, ch + 1) if ch + 1 < NCH else (b + 1, 0)
                        n0 = nb_ * S + nch * CH
                        xna = xtp.tile([128, NEH, CH], BF16, tag="xta",
                                       name="xna")
                        nc.sync.dma_start(xna[:], xT_r[:, 0:NEH, n0:n0 + CH])
                        xnb = xtp.tile([128, NEH, CH], BF16, tag="xtb",
                                       name="xnb")
                        nc.sync.dma_start(xnb[:], xT_r[:, NEH:NE, n0:n0 + CH])
                        xpre = ((nb_, nch), xna, xnb)
                    else:
                        xpre = None

                    def xslice(et, lo=None, hi=None):
                        t = xta if et < NEH else xtb
                        e = et if et < NEH else et - NEH
                        if lo is None:
                            return t[:, e, :]
                        return t[:, e, lo:hi]

                    for h in range(hpc):
                        qp = ps_mm.tile([128, CH], F32, tag="qkvp")
                        for et in range(NE):
                            nc.tensor.matmul(
                                qp[:], wq_sb[:, et, h * DH:(h + 1) * DH],
                                xslice(et),
                                start=(et == 0), stop=(et == NE - 1))
                        nc.scalar.activation(qT[:, h, c0:c0 + CH], qp[:],
                                             AF.Identity, scale=scale)
                        pop_proj()
                        kp = ps_mm.tile([128, CH], F32, tag="qkvp")
                        for et in range(NE):
                            nc.tensor.matmul(
                                kp[:], wk_sb[:, et, h * DH:(h + 1) * DH],
                                xslice(et),
                                start=(et == 0), stop=(et == NE - 1))
                        nc.scalar.activation(kT[:, h, c0:c0 + CH], kp[:],
                                             AF.Identity)
                        pop_proj()
                    for st in range(KPC):
                        vp = ps_mm.tile([128, DHC], F32, tag="qkvp")
                        for et in range(NE):
                            nc.tensor.matmul(
                                vp[:], xslice(et, st * 128, (st + 1) * 128),
                                wv_sb[:, et, :],
                                start=(et == 0), stop=(et == NE - 1))
                        nc.scalar.activation(v_sb[:, ch * KPC + st, :], vp[:],
                                             AF.Identity)
                        pop_proj()

                # ------- Phase B: attention, software-pipelined ----------
                # Per (chunk g, head h): scores (transposed), exp, attn@V and
                # ones-denominator accumulate on PE; the scores matmul for
                # k-tile t+1 issues before attn@V of t so Exp hides.  Proj
                # jobs drain one per k-tile step as PE filler.
                ctxn = qkv.tile([128, hpc, S], BF16, tag="ctxn")
                for g in range(NCH):
                    for h in range(hpc):
                        nk = KPC * (g + 1)
                        avp = ps_av.tile([128, CH], F32, tag="av")
                        dnp = ps_den.tile([1, CH], F32, tag="den")
                        sps = [None] * nk
                        exs = [None] * nk
                        offs = [0] * nk

                        def emit_sp_exp(kt):
                            j = kt - (nk - KPC)
                            off = 128 * j if j > 0 else 0
                            w = CH - off
                            offs[kt] = off
                            sp = ps_mm.tile([128, CH], F32, tag="qkvp",
                                            name="sp")
                            nc.tensor.matmul(
                                sp[:, off:], kT[:, h, kt * 128:(kt + 1) * 128],
                                qT[:, h, g * CH + off:(g + 1) * CH],
                                start=True, stop=True)
                            if j >= 0:
                                nc.vector.tensor_add(sp[:, off:], sp[:, off:],
                                                     mask_sb[:, 0:w])
                            ex = expp.tile([128, CH], BF16, tag="ex")
                            nc.scalar.activation(ex[:, off:], sp[:, off:],
                                                 AF.Exp)
                            sps[kt] = sp
                            exs[kt] = ex

                        def emit_av_dn(kt):
                            off = offs[kt]
                            nc.tensor.matmul(
                                avp[:, off:],
                                v_sb[:, kt, h * DH:(h + 1) * DH],
                                exs[kt][:, off:],
                                start=(kt == 0), stop=(kt == nk - 1),
                                skip_group_check=True)
                            nc.tensor.matmul(
                                dnp[:, off:], ones_sb[:], exs[kt][:, off:],
                                start=(kt == 0), stop=(kt == nk - 1),
                                skip_group_check=True)
                            exs[kt] = None

                        for kt in range(nk):
                            emit_sp_exp(kt)
                            if kt >= 1:
                                emit_av_dn(kt - 1)
                                pop_proj()
                        emit_av_dn(nk - 1)

                        # denominator -> 1/den broadcast across partitions,
                        # applied at the ctx eviction (per-column scale)
                        rrow = denp_sb.tile([1, CH], F32, tag="rrow")
                        nc.vector.reciprocal(rrow[:], dnp[:])
                        rbc = denp_sb.tile([128, CH], F32, tag="rbc")
                        nc.gpsimd.partition_broadcast(rbc[:], rrow[:])
                        nc.vector.tensor_tensor(
                            ctxn[:, h, g * CH:(g + 1) * CH], avp[:], rbc[:],
                            op=ALU.mult)
                    for st in range(g * KPC, (g + 1) * KPC):
                        for oc in range(NOC):
                            proj_jobs.append((ctxn, s0, st, oc))
            pop_proj(len(proj_jobs))
    nc.finalize()
    return nc


def host_consts(S=S_FULL, CH=512):
    """Mask / ones constant inputs."""
    p = np.arange(128)[:, None]
    c = np.arange(CH)[None, :]
    # strict lower triangle: masked iff c < p (diagonal k-tile suffix mask)
    masks = np.where(c < p, np.float32(NEG), np.float32(0.0))
    masks = np.ascontiguousarray(masks.astype(ml_dtypes.bfloat16))
    return {
        "masks": masks,
        "ones": np.ones((128, 1), dtype=ml_dtypes.bfloat16),
    }


def host_inputs(x, Wq, Wk, Wv, Wo, B=B_FULL, S=S_FULL, E=EMB, hpc=HPC,
                DH=HEAD_DIM, CH=512):
    """Shard + lay out the full inputs for the 8 cores (bf16 on device)."""
    SB = B * S
    DHC = hpc * DH
    bf = ml_dtypes.bfloat16
    xT = np.ascontiguousarray(x.reshape(SB, E).T.astype(bf))
    consts = host_consts(S, CH)

    in_maps = []
    for c in range(N_CORES):
        lo, hi = c * DHC, (c + 1) * DHC
        in_maps.append({
            "xT": xT,
            "wqT": np.ascontiguousarray(Wq[lo:hi, :].T.astype(bf)),
            "wkT": np.ascontiguousarray(Wk[lo:hi, :].T.astype(bf)),
            "wvT": np.ascontiguousarray(Wv[lo:hi, :].T.astype(bf)),
            "woT": np.ascontiguousarray(Wo[:, lo:hi].T.astype(bf)),
            **consts,
        })
    return in_maps


def kernel(x, Wq, Wk, Wv, Wo, bo):
    x = np.asarray(x, dtype=np.float32)
    Wq = np.asarray(Wq, dtype=np.float32)
    Wk = np.asarray(Wk, dtype=np.float32)
    Wv = np.asarray(Wv, dtype=np.float32)
    Wo = np.asarray(Wo, dtype=np.float32)
    bo = np.asarray(bo, dtype=np.float32)

    nc = build()
    in_maps = host_inputs(x, Wq, Wk, Wv, Wo)
    res = run_bass_kernel_spmd(nc, in_maps, list(range(N_CORES)))
    y = res.results[0]["y"].astype(np.float32)
    for c in range(1, N_CORES):
        y += res.results[c]["y"].astype(np.float32)
    y = (y + bo).astype(np.float32)
    return y.reshape(B_FULL, S_FULL, EMB)


# revision 8
# speedup vs baseline: 1.5482x; 1.2087x over previous
"""Trainium2 Bass kernel: cached causal self-attention (dense transformer block).

Full module: y = CausalAttn(x; Wq, Wk, Wv) @ Wo.T + bo with
  B=4, S=2048, E=2048, H=16 heads, Dh=128, fp32 inputs.

Distribution: 8-way tensor parallel over heads (2 heads per NeuronCore).
Each core computes Q/K/V projections for its 2 heads, causal-softmax
attention for those heads, and a partial output projection
y_c = ctx_c @ Wo[:, c*256:(c+1)*256].T.  The host sums the 8 partials and
adds the bias, avoiding on-device collectives.

v2 changes vs the fp32r baseline:
  - all matmul operands in bf16 (PSUM accumulation stays fp32): halves
    SBUF/HBM traffic and PE power (the fp32r version hit sustained power
    throttling, ~1.8 GHz effective vs 2.4 GHz peak).
  - attention inner loop software-pipelined: the scores matmul for k-tile
    t+1 issues before attn@V of tile t, so the Exp activation latency hides
    under PE work instead of stalling the accumulation chain.
  - output-projection matmuls run as a job queue drained one tile per
    k-tile step (and between QKV chains), filling the remaining PE bubbles
    of the attention dependency chain.
  - softmax normalization moved off the y path: 1/denominator is broadcast
    across partitions once per (chunk, head) and applied at the ctx PSUM
    eviction, so both heads' projection partials accumulate in one PSUM
    bank and evict with a single copy (the baseline spent 2-3 Vector ops
    per y tile on it).  The denominator transpose DRAM bounce is gone.
  - y partials transfer in bf16 (host sums in fp32).

v3 changes vs v2:
  - the per-k-tile ones-matmul denominator chains came off the PE: exp
    tiles accumulate on the Vector engine (bf16 adds; the final 128-way
    partition sum still happens in fp32 on the PE, so the rounding error
    of the running sums averages out ~1/sqrt(128)), and a single
    ones-by-acc matmul per (chunk, head) produces the denominator row.
  - 1/denominator via reciprocal_approx_fast (the exact Vector reciprocal
    measured 3.3us per row and stalled the next unit's PSUM bank reuse).
  - the denominator->broadcast->ctx-evict chain is deferred into the next
    unit's k-tile loop, so its latency rides under independent PE work.
"""

import math

import ml_dtypes
import numpy as np

import concourse.bacc as bacc
import concourse.mybir as mybir
import concourse.tile as tile
from concourse.bass_utils import run_bass_kernel_spmd

F32 = mybir.dt.float32
BF16 = mybir.dt.bfloat16
AF = mybir.ActivationFunctionType
ALU = mybir.AluOpType

NEG = -1.0e30

# Full-problem constants
EMB = 2048
N_HEADS = 16
HEAD_DIM = 128
B_FULL = 4
S_FULL = 2048
N_CORES = 8
HPC = N_HEADS // N_CORES  # heads per core = 2


def build(B=B_FULL, S=S_FULL, E=EMB, hpc=HPC, DH=HEAD_DIM, CH=512):
    """Build the per-core Bass program (same program on all 8 cores)."""
    assert hpc == 2, "projection accumulation is written for 2 heads per core"
    SB = B * S
    DHC = hpc * DH          # per-core head dims (256)
    NE = E // 128           # e-tiles (contraction tiles)
    NEH = NE // 2
    NCH = S // CH           # 512-wide chunks per sequence
    KPC = CH // 128         # k-tiles per chunk (4)
    NST = S // 128          # 128-row s-tiles per sequence
    NOC = E // CH           # output chunks
    scale = 1.0 / math.sqrt(DH)

    nc = bacc.Bacc("TRN2", target_bir_lowering=False, debug=False,
                   num_devices=N_CORES)

    xT = nc.dram_tensor("xT", [E, SB], BF16, kind="ExternalInput")
    wqT = nc.dram_tensor("wqT", [E, DHC], BF16, kind="ExternalInput")
    wkT = nc.dram_tensor("wkT", [E, DHC], BF16, kind="ExternalInput")
    wvT = nc.dram_tensor("wvT", [E, DHC], BF16, kind="ExternalInput")
    woT = nc.dram_tensor("woT", [DHC, E], BF16, kind="ExternalInput")
    masks = nc.dram_tensor("masks", [128, CH], BF16, kind="ExternalInput")
    ones = nc.dram_tensor("ones", [128, 1], BF16, kind="ExternalInput")
    y = nc.dram_tensor("y", [SB, E], BF16, kind="ExternalOutput")

    with tile.TileContext(nc) as tc:
        with (
            tc.tile_pool(name="wpool", bufs=1) as wpool,
            tc.tile_pool(name="xtp", bufs=2) as xtp,
            tc.tile_pool(name="qkv", bufs=1) as qkv,
            tc.tile_pool(name="expp", bufs=4) as expp,
            tc.tile_pool(name="accp", bufs=2) as accp,
            tc.tile_pool(name="denp", bufs=2) as denp_sb,
            tc.tile_pool(name="yout", bufs=3) as yout,
            tc.tile_pool(name="ps_mm", bufs=3, space="PSUM") as ps_mm,
            tc.tile_pool(name="ps_proj", bufs=2, space="PSUM") as ps_proj,
            tc.tile_pool(name="ps_av", bufs=2, space="PSUM") as ps_av,
            tc.tile_pool(name="ps_den", bufs=1, space="PSUM") as ps_den,
        ):
            # Resident weights / constants.  The first Q chain consumes
            # e-tiles in order, so stage the DMAs so its head-of-stream
            # tiles (wq + x quarter 0) land first.
            wq_sb = wpool.tile([128, NE, DHC], BF16, tag="wq")
            wk_sb = wpool.tile([128, NE, DHC], BF16, tag="wk")
            wv_sb = wpool.tile([128, NE, DHC], BF16, tag="wv")
            wo_sb = wpool.tile([128, hpc, E], BF16, tag="wo")
            xT_r = xT.rearrange("(t p) s -> p t s", p=128)
            wq_r = wqT.rearrange("(t p) d -> p t d", p=128)
            wk_r = wkT.rearrange("(t p) d -> p t d", p=128)

            nc.sync.dma_start(wq_sb[:, 0:4, :], wq_r[:, 0:4, :])
            x0a = xtp.tile([128, NEH, CH], BF16, tag="xta", name="x0a")
            x0b = xtp.tile([128, NEH, CH], BF16, tag="xtb", name="x0b")
            nc.sync.dma_start(x0a[:, 0:4, :], xT_r[:, 0:4, 0:CH])
            nc.sync.dma_start(wq_sb[:, 4:NEH, :], wq_r[:, 4:NEH, :])
            nc.sync.dma_start(x0a[:, 4:NEH, :], xT_r[:, 4:NEH, 0:CH])
            nc.sync.dma_start(wq_sb[:, NEH:NE, :], wq_r[:, NEH:NE, :])
            nc.sync.dma_start(x0b[:], xT_r[:, NEH:NE, 0:CH])
            nc.sync.dma_start(wk_sb[:, 0:NEH, :], wk_r[:, 0:NEH, :])
            nc.sync.dma_start(wk_sb[:, NEH:NE, :], wk_r[:, NEH:NE, :])
            xpre0 = ((0, 0), x0a, x0b)
            nc.sync.dma_start(wv_sb[:], wvT.rearrange("(t p) d -> p t d", p=128))
            nc.sync.dma_start(wo_sb[:], woT.rearrange("(h p) e -> p h e", p=128))
            mask_sb = wpool.tile([128, CH], BF16, tag="mask")
            nc.sync.dma_start(mask_sb[:], masks[:, :])
            ones_sb = wpool.tile([128, 1], BF16, tag="ones")
            nc.sync.dma_start(ones_sb[:], ones[:, :])

            # ---- output-projection job queue --------------------------
            # A job is one y tile (st, oc): both heads' partials accumulate
            # into one PSUM tile, single-op eviction (alternating engine),
            # DMA from the Sync engine (Pool stays light).
            proj_jobs = []
            evict_flip = [0]
            pending_den = [None]

            def drain_den():
                if pending_den[0] is not None:
                    job, pending_den[0] = pending_den[0], None
                    job()

            def emit_proj_job(job):
                ctxn, s0, st, oc = job
                o0 = oc * CH
                p = ps_proj.tile([128, CH], F32, tag="proj")
                nc.tensor.matmul(p[:], ctxn[:, 0, st * 128:(st + 1) * 128],
                                 wo_sb[:, 0, o0:o0 + CH],
                                 start=True, stop=False)
                nc.tensor.matmul(p[:], ctxn[:, 1, st * 128:(st + 1) * 128],
                                 wo_sb[:, 1, o0:o0 + CH],
                                 start=False, stop=True)
                ysb = yout.tile([128, CH], BF16, tag="ysb")
                if evict_flip[0] == 0:
                    nc.scalar.copy(ysb[:], p[:])
                else:
                    nc.vector.tensor_copy(ysb[:], p[:])
                evict_flip[0] ^= 1
                nc.sync.dma_start(
                    y[s0 + st * 128:s0 + (st + 1) * 128, o0:o0 + CH], ysb[:])

            def pop_proj(n=1):
                for _ in range(min(n, len(proj_jobs))):
                    emit_proj_job(proj_jobs.pop(0))

            xpre = xpre0
            for b in range(B):
                s0 = b * S
                # ---------------- Phase A: Q/K/V projections -------------
                qT = qkv.tile([128, hpc, S], BF16, tag="qT")
                kT = qkv.tile([128, hpc, S], BF16, tag="kT")
                v_sb = qkv.tile([128, NST, DHC], BF16, tag="v")
                for ch in range(NCH):
                    c0 = ch * CH
                    if xpre is not None and xpre[0] == (b, ch):
                        xta, xtb = xpre[1], xpre[2]
                    else:
                        xta = xtp.tile([128, NEH, CH], BF16, tag="xta")
                        nc.sync.dma_start(xta[:],
                                          xT_r[:, 0:NEH, s0 + c0:s0 + c0 + CH])
                        xtb = xtp.tile([128, NEH, CH], BF16, tag="xtb")
                        nc.sync.dma_start(xtb[:],
                                          xT_r[:, NEH:NE, s0 + c0:s0 + c0 + CH])
                    if ch + 1 < NCH or b + 1 < B:
                        nb_, nch = (b, ch + 1) if ch + 1 < NCH else (b + 1, 0)
                        n0 = nb_ * S + nch * CH
                        xna = xtp.tile([128, NEH, CH], BF16, tag="xta",
                                       name="xna")
                        nc.sync.dma_start(xna[:], xT_r[:, 0:NEH, n0:n0 + CH])
                        xnb = xtp.tile([128, NEH, CH], BF16, tag="xtb",
                                       name="xnb")
                        nc.sync.dma_start(xnb[:], xT_r[:, NEH:NE, n0:n0 + CH])
                        xpre = ((nb_, nch), xna, xnb)
                    else:
                        xpre = None

                    def xslice(et, lo=None, hi=None):
                        t = xta if et < NEH else xtb
                        e = et if et < NEH else et - NEH
                        if lo is None:
                            return t[:, e, :]
                        return t[:, e, lo:hi]

                    for h in range(hpc):
                        qp = ps_mm.tile([128, CH], F32, tag="qkvp")
                        for et in range(NE):
                            nc.tensor.matmul(
                                qp[:], wq_sb[:, et, h * DH:(h + 1) * DH],
                                xslice(et),
                                start=(et == 0), stop=(et == NE - 1))
                        nc.scalar.activation(qT[:, h, c0:c0 + CH], qp[:],
                                             AF.Identity, scale=scale)
                        drain_den()
                        pop_proj()
                        kp = ps_mm.tile([128, CH], F32, tag="qkvp")
                        for et in range(NE):
                            nc.tensor.matmul(
                                kp[:], wk_sb[:, et, h * DH:(h + 1) * DH],
                                xslice(et),
                                start=(et == 0), stop=(et == NE - 1))
                        nc.scalar.activation(kT[:, h, c0:c0 + CH], kp[:],
                                             AF.Identity)
                        pop_proj()
                    for st in range(KPC):
                        vp = ps_mm.tile([128, DHC], F32, tag="qkvp")
                        for et in range(NE):
                            nc.tensor.matmul(
                                vp[:], xslice(et, st * 128, (st + 1) * 128),
                                wv_sb[:, et, :],
                                start=(et == 0), stop=(et == NE - 1))
                        nc.scalar.activation(v_sb[:, ch * KPC + st, :], vp[:],
                                             AF.Identity)
                        pop_proj()

                # Flush any leftover proj jobs of the previous batch before
                # its ctxn buffer is recycled below.
                pop_proj(len(proj_jobs))

                # ------- Phase B: attention, software-pipelined ----------
                # Per (chunk g, head h): scores (transposed) and attn@V on
                # PE; the scores matmul for k-tile t+1 issues before attn@V
                # of t so Exp hides.  Exp tiles also accumulate on Vector
                # (bf16) into acc; the denominator chain (ones-by-acc
                # matmul, approx reciprocal, partition broadcast, normalized
                # ctx eviction) is deferred into the NEXT unit's k-tile loop
                # so none of its latency blocks the PE.  Proj jobs drain one
                # per k-tile step as PE filler.
                ctxn = qkv.tile([128, hpc, S], BF16, tag="ctxn")
                for g in range(NCH):
                    for h in range(hpc):
                        nk = KPC * (g + 1)
                        avp = ps_av.tile([128, CH], F32, tag="av")
                        acc = accp.tile([128, CH], BF16, tag="acc")
                        exs = [None] * nk
                        offs = [0] * nk

                        def emit_sp_exp(kt):
                            j = kt - (nk - KPC)
                            off = 128 * j if j > 0 else 0
                            w = CH - off
                            offs[kt] = off
                            sp = ps_mm.tile([128, CH], F32, tag="qkvp",
                                            name="sp")
                            nc.tensor.matmul(
                                sp[:, off:], kT[:, h, kt * 128:(kt + 1) * 128],
                                qT[:, h, g * CH + off:(g + 1) * CH],
                                start=True, stop=True)
                            if j >= 0:
                                nc.vector.tensor_add(sp[:, off:], sp[:, off:],
                                                     mask_sb[:, 0:w])
                            ex = expp.tile([128, CH], BF16, tag="ex")
                            nc.scalar.activation(ex[:, off:], sp[:, off:],
                                                 AF.Exp)
                            exs[kt] = ex

                        def emit_av_acc(kt):
                            off = offs[kt]
                            nc.tensor.matmul(
                                avp[:, off:],
                                v_sb[:, kt, h * DH:(h + 1) * DH],
                                exs[kt][:, off:],
                                start=(kt == 0), stop=(kt == nk - 1),
                                skip_group_check=True)
                            if kt == 0:
                                nc.vector.tensor_copy(acc[:], exs[kt][:])
                            else:
                                nc.vector.tensor_add(acc[:, off:],
                                                     acc[:, off:],
                                                     exs[kt][:, off:])
                            exs[kt] = None

                        for kt in range(nk):
                            emit_sp_exp(kt)
                            if kt == 1:
                                drain_den()
                            if kt >= 1:
                                emit_av_acc(kt - 1)
                                pop_proj()
                        emit_av_acc(nk - 1)

                        def den_job(g=g, h=h, avp=avp, acc=acc, ctxn=ctxn,
                                    s0=s0):
                            dnp = ps_den.tile([1, CH], F32, tag="den")
                            nc.tensor.matmul(dnp[:], ones_sb[:], acc[:],
                                             start=True, stop=True)
                            rrow = denp_sb.tile([1, CH], F32, tag="rrow")
                            nc.vector.reciprocal_approx_fast(rrow[:], dnp[:])
                            rbc = denp_sb.tile([128, CH], F32, tag="rbc")
                            nc.gpsimd.partition_broadcast(rbc[:], rrow[:])
                            nc.vector.tensor_tensor(
                                ctxn[:, h, g * CH:(g + 1) * CH], avp[:],
                                rbc[:], op=ALU.mult)
                            if h == 1:
                                for st in range(g * KPC, (g + 1) * KPC):
                                    for oc in range(NOC):
                                        proj_jobs.append((ctxn, s0, st, oc))
                        pending_den[0] = den_job
            drain_den()
            pop_proj(len(proj_jobs))
    nc.finalize()
    return nc


def host_consts(S=S_FULL, CH=512):
    """Mask / ones constant inputs."""
    p = np.arange(128)[:, None]
    c = np.arange(CH)[None, :]
    # strict lower triangle: masked iff c < p (diagonal k-tile suffix mask)
    masks = np.where(c < p, np.float32(NEG), np.float32(0.0))
    masks = np.ascontiguousarray(masks.astype(ml_dtypes.bfloat16))
    return {
        "masks": masks,
        "ones": np.ones((128, 1), dtype=ml_dtypes.bfloat16),
    }


def host_inputs(x, Wq, Wk, Wv, Wo, B=B_FULL, S=S_FULL, E=EMB, hpc=HPC,
                DH=HEAD_DIM, CH=512):
    """Shard + lay out the full inputs for the 8 cores (bf16 on device)."""
    SB = B * S
    DHC = hpc * DH
    bf = ml_dtypes.bfloat16
    xT = np.ascontiguousarray(x.reshape(SB, E).T.astype(bf))
    consts = host_consts(S, CH)

    in_maps = []
    for c in range(N_CORES):
        lo, hi = c * DHC, (c + 1) * DHC
        in_maps.append({
            "xT": xT,
            "wqT": np.ascontiguousarray(Wq[lo:hi, :].T.astype(bf)),
            "wkT": np.ascontiguousarray(Wk[lo:hi, :].T.astype(bf)),
            "wvT": np.ascontiguousarray(Wv[lo:hi, :].T.astype(bf)),
            "woT": np.ascontiguousarray(Wo[:, lo:hi].T.astype(bf)),
            **consts,
        })
    return in_maps


def kernel(x, Wq, Wk, Wv, Wo, bo):
    x = np.asarray(x, dtype=np.float32)
    Wq = np.asarray(Wq, dtype=np.float32)
    Wk = np.asarray(Wk, dtype=np.float32)
    Wv = np.asarray(Wv, dtype=np.float32)
    Wo = np.asarray(Wo, dtype=np.float32)
    bo = np.asarray(bo, dtype=np.float32)

    nc = build()
    in_maps = host_inputs(x, Wq, Wk, Wv, Wo)
    res = run_bass_kernel_spmd(nc, in_maps, list(range(N_CORES)))
    y = res.results[0]["y"].astype(np.float32)
    for c in range(1, N_CORES):
        y += res.results[c]["y"].astype(np.float32)
    y = (y + bo).astype(np.float32)
    return y.reshape(B_FULL, S_FULL, EMB)


# revision 11
# speedup vs baseline: 1.5812x; 1.0213x over previous
"""Trainium2 Bass kernel: cached causal self-attention (dense transformer block).

Full module: y = CausalAttn(x; Wq, Wk, Wv) @ Wo.T + bo with
  B=4, S=2048, E=2048, H=16 heads, Dh=128, fp32 inputs.

Distribution: 8-way tensor parallel over heads (2 heads per NeuronCore).
Each core computes Q/K/V projections for its 2 heads, causal-softmax
attention for those heads, and a partial output projection
y_c = ctx_c @ Wo[:, c*256:(c+1)*256].T.  The host sums the 8 partials and
adds the bias, avoiding on-device collectives.

v2 changes vs the fp32r baseline:
  - all matmul operands in bf16 (PSUM accumulation stays fp32): halves
    SBUF/HBM traffic and PE power (the fp32r version hit sustained power
    throttling, ~1.8 GHz effective vs 2.4 GHz peak).
  - attention inner loop software-pipelined: the scores matmul for k-tile
    t+1 issues before attn@V of tile t, so the Exp activation latency hides
    under PE work instead of stalling the accumulation chain.
  - output-projection matmuls run as a job queue drained one tile per
    k-tile step (and between QKV chains), filling the remaining PE bubbles
    of the attention dependency chain.
  - softmax normalization moved off the y path: 1/denominator is broadcast
    across partitions once per (chunk, head) and applied at the ctx PSUM
    eviction, so both heads' projection partials accumulate in one PSUM
    bank and evict with a single copy (the baseline spent 2-3 Vector ops
    per y tile on it).  The denominator transpose DRAM bounce is gone.
  - y partials transfer in bf16 (host sums in fp32).

v3 changes vs v2:
  - the per-k-tile ones-matmul denominator chains came off the PE: exp
    tiles accumulate on the Vector engine (bf16 adds; the final 128-way
    partition sum still happens in fp32 on the PE, so the rounding error
    of the running sums averages out ~1/sqrt(128)), and a single
    ones-by-acc matmul per (chunk, head) produces the denominator row.
  - 1/denominator via reciprocal_approx_fast (the exact Vector reciprocal
    measured 3.3us per row and stalled the next unit's PSUM bank reuse).
  - the denominator->broadcast->ctx-evict chain is deferred into the next
    unit's k-tile loop, so its latency rides under independent PE work.
"""

import math

import ml_dtypes
import numpy as np

import concourse.bacc as bacc
import concourse.mybir as mybir
import concourse.tile as tile
from concourse.bass_utils import run_bass_kernel_spmd

F32 = mybir.dt.float32
BF16 = mybir.dt.bfloat16
AF = mybir.ActivationFunctionType
ALU = mybir.AluOpType

NEG = -1.0e30

# Full-problem constants
EMB = 2048
N_HEADS = 16
HEAD_DIM = 128
B_FULL = 4
S_FULL = 2048
N_CORES = 8
HPC = N_HEADS // N_CORES  # heads per core = 2


def build(B=B_FULL, S=S_FULL, E=EMB, hpc=HPC, DH=HEAD_DIM, CH=512):
    """Build the per-core Bass program (same program on all 8 cores)."""
    assert hpc == 2, "projection accumulation is written for 2 heads per core"
    SB = B * S
    DHC = hpc * DH          # per-core head dims (256)
    NE = E // 128           # e-tiles (contraction tiles)
    NEH = NE // 2
    NCH = S // CH           # 512-wide chunks per sequence
    KPC = CH // 128         # k-tiles per chunk (4)
    NST = S // 128          # 128-row s-tiles per sequence
    NOC = E // CH           # output chunks
    scale = 1.0 / math.sqrt(DH)

    nc = bacc.Bacc("TRN2", target_bir_lowering=False, debug=False,
                   num_devices=N_CORES)

    xT = nc.dram_tensor("xT", [E, SB], BF16, kind="ExternalInput")
    wqT = nc.dram_tensor("wqT", [E, DHC], BF16, kind="ExternalInput")
    wkT = nc.dram_tensor("wkT", [E, DHC], BF16, kind="ExternalInput")
    wvT = nc.dram_tensor("wvT", [E, DHC], BF16, kind="ExternalInput")
    woT = nc.dram_tensor("woT", [DHC, E], BF16, kind="ExternalInput")
    masks = nc.dram_tensor("masks", [128, CH], BF16, kind="ExternalInput")
    ones = nc.dram_tensor("ones", [128, 1], BF16, kind="ExternalInput")
    y = nc.dram_tensor("y", [SB, E], BF16, kind="ExternalOutput")

    with tile.TileContext(nc) as tc:
        with (
            tc.tile_pool(name="wpool", bufs=1) as wpool,
            tc.tile_pool(name="xtp", bufs=2) as xtp,
            tc.tile_pool(name="qkv", bufs=1) as qkv,
            tc.tile_pool(name="expp", bufs=4) as expp,
            tc.tile_pool(name="accp", bufs=2) as accp,
            tc.tile_pool(name="denp", bufs=2) as denp_sb,
            tc.tile_pool(name="yout", bufs=3) as yout,
            tc.tile_pool(name="ps_mm", bufs=3, space="PSUM") as ps_mm,
            tc.tile_pool(name="ps_proj", bufs=2, space="PSUM") as ps_proj,
            tc.tile_pool(name="ps_av", bufs=2, space="PSUM") as ps_av,
            tc.tile_pool(name="ps_den", bufs=1, space="PSUM") as ps_den,
        ):
            # Resident weights / constants.  The first Q chain consumes
            # e-tiles in order, so stage the DMAs so its head-of-stream
            # tiles (wq + x quarter 0) land first.
            wq_sb = wpool.tile([128, NE, DHC], BF16, tag="wq")
            wk_sb = wpool.tile([128, NE, DHC], BF16, tag="wk")
            wv_sb = wpool.tile([128, NE, DHC], BF16, tag="wv")
            wo_sb = wpool.tile([128, hpc, E], BF16, tag="wo")
            xT_r = xT.rearrange("(t p) s -> p t s", p=128)
            wq_r = wqT.rearrange("(t p) d -> p t d", p=128)
            wk_r = wkT.rearrange("(t p) d -> p t d", p=128)

            x0a = xtp.tile([128, NEH, CH], BF16, tag="xta", name="x0a")
            x0b = xtp.tile([128, NEH, CH], BF16, tag="xtb", name="x0b")
            nc.sync.dma_start(wq_sb[:, 0:2, :], wq_r[:, 0:2, :])
            nc.sync.dma_start(x0a[:, 0:2, :], xT_r[:, 0:2, 0:CH])
            nc.sync.dma_start(wq_sb[:, 2:4, :], wq_r[:, 2:4, :])
            nc.sync.dma_start(x0a[:, 2:4, :], xT_r[:, 2:4, 0:CH])
            nc.sync.dma_start(wq_sb[:, 4:NEH, :], wq_r[:, 4:NEH, :])
            nc.sync.dma_start(x0a[:, 4:NEH, :], xT_r[:, 4:NEH, 0:CH])
            nc.sync.dma_start(wq_sb[:, NEH:12, :], wq_r[:, NEH:12, :])
            nc.sync.dma_start(x0b[:, 0:4, :], xT_r[:, NEH:12, 0:CH])
            nc.sync.dma_start(wq_sb[:, 12:NE, :], wq_r[:, 12:NE, :])
            nc.sync.dma_start(x0b[:, 4:NEH, :], xT_r[:, 12:NE, 0:CH])
            nc.sync.dma_start(wk_sb[:, 0:NEH, :], wk_r[:, 0:NEH, :])
            nc.sync.dma_start(wk_sb[:, NEH:NE, :], wk_r[:, NEH:NE, :])
            xpre0 = ((0, 0), x0a, x0b)
            nc.sync.dma_start(wv_sb[:], wvT.rearrange("(t p) d -> p t d", p=128))
            nc.sync.dma_start(wo_sb[:], woT.rearrange("(h p) e -> p h e", p=128))
            mask_sb = wpool.tile([128, CH], BF16, tag="mask")
            nc.sync.dma_start(mask_sb[:], masks[:, :])
            ones_sb = wpool.tile([128, 1], BF16, tag="ones")
            nc.sync.dma_start(ones_sb[:], ones[:, :])

            # ---- output-projection job queue --------------------------
            # A job is one y tile (st, oc): both heads' partials accumulate
            # into one PSUM tile, single-op eviction (alternating engine),
            # DMA from the Sync engine (Pool stays light).
            proj_jobs = []
            evict_flip = [0]
            pending_den = [None]

            def drain_den():
                if pending_den[0] is not None:
                    job, pending_den[0] = pending_den[0], None
                    job()

            def emit_proj_job(job):
                ctxn, s0, st, oc = job
                o0 = oc * CH
                p = ps_proj.tile([128, CH], F32, tag="proj")
                nc.tensor.matmul(p[:], ctxn[:, 0, st * 128:(st + 1) * 128],
                                 wo_sb[:, 0, o0:o0 + CH],
                                 start=True, stop=False)
                nc.tensor.matmul(p[:], ctxn[:, 1, st * 128:(st + 1) * 128],
                                 wo_sb[:, 1, o0:o0 + CH],
                                 start=False, stop=True)
                ysb = yout.tile([128, CH], BF16, tag="ysb")
                if evict_flip[0] == 0:
                    nc.scalar.copy(ysb[:], p[:])
                else:
                    nc.vector.tensor_copy(ysb[:], p[:])
                evict_flip[0] ^= 1
                nc.sync.dma_start(
                    y[s0 + st * 128:s0 + (st + 1) * 128, o0:o0 + CH], ysb[:])

            def pop_proj(n=1):
                for _ in range(min(n, len(proj_jobs))):
                    emit_proj_job(proj_jobs.pop(0))

            xpre = xpre0
            for b in range(B):
                s0 = b * S
                # ---------------- Phase A: Q/K/V projections -------------
                qT = qkv.tile([128, hpc, S], BF16, tag="qT")
                kT = qkv.tile([128, hpc, S], BF16, tag="kT")
                v_sb = qkv.tile([128, NST, DHC], BF16, tag="v")
                for ch in range(NCH):
                    c0 = ch * CH
                    if xpre is not None and xpre[0] == (b, ch):
                        xta, xtb = xpre[1], xpre[2]
                    else:
                        xta = xtp.tile([128, NEH, CH], BF16, tag="xta")
                        nc.sync.dma_start(xta[:],
                                          xT_r[:, 0:NEH, s0 + c0:s0 + c0 + CH])
                        xtb = xtp.tile([128, NEH, CH], BF16, tag="xtb")
                        nc.sync.dma_start(xtb[:],
                                          xT_r[:, NEH:NE, s0 + c0:s0 + c0 + CH])
                    if ch + 1 < NCH or b + 1 < B:
                        nb_, nch = (b, ch + 1) if ch + 1 < NCH else (b + 1, 0)
                        n0 = nb_ * S + nch * CH
                        xna = xtp.tile([128, NEH, CH], BF16, tag="xta",
                                       name="xna")
                        nc.sync.dma_start(xna[:], xT_r[:, 0:NEH, n0:n0 + CH])
                        xnb = xtp.tile([128, NEH, CH], BF16, tag="xtb",
                                       name="xnb")
                        nc.sync.dma_start(xnb[:], xT_r[:, NEH:NE, n0:n0 + CH])
                        xpre = ((nb_, nch), xna, xnb)
                    else:
                        xpre = None

                    def xslice(et, lo=None, hi=None):
                        t = xta if et < NEH else xtb
                        e = et if et < NEH else et - NEH
                        if lo is None:
                            return t[:, e, :]
                        return t[:, e, lo:hi]

                    for h in range(hpc):
                        qp = ps_mm.tile([128, CH], F32, tag="qkvp")
                        for et in range(NE):
                            nc.tensor.matmul(
                                qp[:], wq_sb[:, et, h * DH:(h + 1) * DH],
                                xslice(et),
                                start=(et == 0), stop=(et == NE - 1))
                        nc.scalar.activation(qT[:, h, c0:c0 + CH], qp[:],
                                             AF.Identity, scale=scale)
                        drain_den()
                        pop_proj()
                        kp = ps_mm.tile([128, CH], F32, tag="qkvp")
                        for et in range(NE):
                            nc.tensor.matmul(
                                kp[:], wk_sb[:, et, h * DH:(h + 1) * DH],
                                xslice(et),
                                start=(et == 0), stop=(et == NE - 1))
                        nc.scalar.activation(kT[:, h, c0:c0 + CH], kp[:],
                                             AF.Identity)
                        pop_proj()
                    for st in range(KPC):
                        vp = ps_mm.tile([128, DHC], F32, tag="qkvp")
                        for et in range(NE):
                            nc.tensor.matmul(
                                vp[:], xslice(et, st * 128, (st + 1) * 128),
                                wv_sb[:, et, :],
                                start=(et == 0), stop=(et == NE - 1))
                        nc.scalar.activation(v_sb[:, ch * KPC + st, :], vp[:],
                                             AF.Identity)
                        pop_proj()

                # Flush any leftover proj jobs of the previous batch before
                # its ctxn buffer is recycled below.
                pop_proj(len(proj_jobs))

                # ------- Phase B: attention, software-pipelined ----------
                # Per (chunk g, head h): scores (transposed) and attn@V on
                # PE; the scores matmul for k-tile t+1 issues before attn@V
                # of t so Exp hides.  Exp tiles also accumulate on Vector
                # (bf16) into acc; the denominator chain (ones-by-acc
                # matmul, approx reciprocal, partition broadcast, normalized
                # ctx eviction) is deferred into the NEXT unit's k-tile loop
                # so none of its latency blocks the PE.  Proj jobs drain one
                # per k-tile step as PE filler.
                ctxn = qkv.tile([128, hpc, S], BF16, tag="ctxn")
                for g in range(NCH):
                    for h in range(hpc):
                        nk = KPC * (g + 1)
                        avp = ps_av.tile([128, CH], F32, tag="av")
                        acc = accp.tile([128, CH], BF16, tag="acc")
                        exs = [None] * nk
                        offs = [0] * nk

                        def emit_sp_exp(kt):
                            j = kt - (nk - KPC)
                            off = 128 * j if j > 0 else 0
                            w = CH - off
                            offs[kt] = off
                            sp = ps_mm.tile([128, CH], F32, tag="qkvp",
                                            name="sp")
                            nc.tensor.matmul(
                                sp[:, off:], kT[:, h, kt * 128:(kt + 1) * 128],
                                qT[:, h, g * CH + off:(g + 1) * CH],
                                start=True, stop=True)
                            if j >= 0:
                                # the triangular mask only spans the first
                                # 128 columns of the suffix; beyond that the
                                # additive mask is all zeros
                                mw = min(w, 128)
                                nc.vector.tensor_add(
                                    sp[:, off:off + mw], sp[:, off:off + mw],
                                    mask_sb[:, 0:mw])
                            ex = expp.tile([128, CH], BF16, tag="ex")
                            nc.scalar.activation(ex[:, off:], sp[:, off:],
                                                 AF.Exp)
                            exs[kt] = ex

                        def emit_av_acc(kt):
                            off = offs[kt]
                            nc.tensor.matmul(
                                avp[:, off:],
                                v_sb[:, kt, h * DH:(h + 1) * DH],
                                exs[kt][:, off:],
                                start=(kt == 0), stop=(kt == nk - 1),
                                skip_group_check=True)
                            if kt == 0:
                                nc.vector.tensor_copy(acc[:], exs[kt][:])
                            else:
                                nc.vector.tensor_add(acc[:, off:],
                                                     acc[:, off:],
                                                     exs[kt][:, off:])
                            exs[kt] = None

                        # lookahead 2: two scores matmuls stay in flight
                        # ahead of attn@V, so the Exp queue lag at unit
                        # start never exposes on the PE
                        for kt in range(nk):
                            emit_sp_exp(kt)
                            if kt == 1:
                                drain_den()
                            if kt >= 2:
                                emit_av_acc(kt - 2)
                                pop_proj()
                        emit_av_acc(nk - 2)
                        pop_proj()
                        emit_av_acc(nk - 1)

                        def den_job(g=g, h=h, avp=avp, acc=acc, ctxn=ctxn,
                                    s0=s0):
                            dnp = ps_den.tile([1, CH], F32, tag="den")
                            nc.tensor.matmul(dnp[:], ones_sb[:], acc[:],
                                             start=True, stop=True)
                            rrow = denp_sb.tile([1, CH], F32, tag="rrow")
                            nc.vector.reciprocal_approx_fast(rrow[:], dnp[:])
                            rbc = denp_sb.tile([128, CH], F32, tag="rbc")
                            nc.gpsimd.partition_broadcast(rbc[:], rrow[:])
                            nc.vector.tensor_tensor(
                                ctxn[:, h, g * CH:(g + 1) * CH], avp[:],
                                rbc[:], op=ALU.mult)
                            if h == 1:
                                for st in range(g * KPC, (g + 1) * KPC):
                                    for oc in range(NOC):
                                        proj_jobs.append((ctxn, s0, st, oc))
                        pending_den[0] = den_job
            drain_den()
            pop_proj(len(proj_jobs))
    nc.finalize()
    return nc


def host_consts(S=S_FULL, CH=512):
    """Mask / ones constant inputs."""
    p = np.arange(128)[:, None]
    c = np.arange(CH)[None, :]
    # strict lower triangle: masked iff c < p (diagonal k-tile suffix mask)
    masks = np.where(c < p, np.float32(NEG), np.float32(0.0))
    masks = np.ascontiguousarray(masks.astype(ml_dtypes.bfloat16))
    return {
        "masks": masks,
        "ones": np.ones((128, 1), dtype=ml_dtypes.bfloat16),
    }


def host_inputs(x, Wq, Wk, Wv, Wo, B=B_FULL, S=S_FULL, E=EMB, hpc=HPC,
                DH=HEAD_DIM, CH=512):
    """Shard + lay out the full inputs for the 8 cores (bf16 on device)."""
    SB = B * S
    DHC = hpc * DH
    bf = ml_dtypes.bfloat16
    xT = np.ascontiguousarray(x.reshape(SB, E).T.astype(bf))
    consts = host_consts(S, CH)

    in_maps = []
    for c in range(N_CORES):
        lo, hi = c * DHC, (c + 1) * DHC
        in_maps.append({
            "xT": xT,
            "wqT": np.ascontiguousarray(Wq[lo:hi, :].T.astype(bf)),
            "wkT": np.ascontiguousarray(Wk[lo:hi, :].T.astype(bf)),
            "wvT": np.ascontiguousarray(Wv[lo:hi, :].T.astype(bf)),
            "woT": np.ascontiguousarray(Wo[:, lo:hi].T.astype(bf)),
            **consts,
        })
    return in_maps


def kernel(x, Wq, Wk, Wv, Wo, bo):
    x = np.asarray(x, dtype=np.float32)
    Wq = np.asarray(Wq, dtype=np.float32)
    Wk = np.asarray(Wk, dtype=np.float32)
    Wv = np.asarray(Wv, dtype=np.float32)
    Wo = np.asarray(Wo, dtype=np.float32)
    bo = np.asarray(bo, dtype=np.float32)

    nc = build()
    in_maps = host_inputs(x, Wq, Wk, Wv, Wo)
    res = run_bass_kernel_spmd(nc, in_maps, list(range(N_CORES)))
    y = res.results[0]["y"].astype(np.float32)
    for c in range(1, N_CORES):
        y += res.results[c]["y"].astype(np.float32)
    y = (y + bo).astype(np.float32)
    return y.reshape(B_FULL, S_FULL, EMB)


# revision 12
# speedup vs baseline: 1.6115x; 1.0192x over previous
"""Trainium2 Bass kernel: cached causal self-attention (dense transformer block).

Full module: y = CausalAttn(x; Wq, Wk, Wv) @ Wo.T + bo with
  B=4, S=2048, E=2048, H=16 heads, Dh=128, fp32 inputs.

Distribution: 8-way tensor parallel over heads (2 heads per NeuronCore).
Each core computes Q/K/V projections for its 2 heads, causal-softmax
attention for those heads, and a partial output projection
y_c = ctx_c @ Wo[:, c*256:(c+1)*256].T.  The host sums the 8 partials and
adds the bias, avoiding on-device collectives.

v2 changes vs the fp32r baseline:
  - all matmul operands in bf16 (PSUM accumulation stays fp32): halves
    SBUF/HBM traffic and PE power (the fp32r version hit sustained power
    throttling, ~1.8 GHz effective vs 2.4 GHz peak).
  - attention inner loop software-pipelined: the scores matmul for k-tile
    t+1 issues before attn@V of tile t, so the Exp activation latency hides
    under PE work instead of stalling the accumulation chain.
  - output-projection matmuls run as a job queue drained one tile per
    k-tile step (and between QKV chains), filling the remaining PE bubbles
    of the attention dependency chain.
  - softmax normalization moved off the y path: 1/denominator is broadcast
    across partitions once per (chunk, head) and applied at the ctx PSUM
    eviction, so both heads' projection partials accumulate in one PSUM
    bank and evict with a single copy (the baseline spent 2-3 Vector ops
    per y tile on it).  The denominator transpose DRAM bounce is gone.
  - y partials transfer in bf16 (host sums in fp32).

v3 changes vs v2:
  - the per-k-tile ones-matmul denominator chains came off the PE: exp
    tiles accumulate on the Vector engine (bf16 adds; the final 128-way
    partition sum still happens in fp32 on the PE, so the rounding error
    of the running sums averages out ~1/sqrt(128)), and a single
    ones-by-acc matmul per (chunk, head) produces the denominator row.
  - 1/denominator via reciprocal_approx_fast (the exact Vector reciprocal
    measured 3.3us per row and stalled the next unit's PSUM bank reuse).
  - the denominator->broadcast->ctx-evict chain is deferred into the next
    unit's k-tile loop, so its latency rides under independent PE work.
"""

import math

import ml_dtypes
import numpy as np

import concourse.bacc as bacc
import concourse.mybir as mybir
import concourse.tile as tile
from concourse.bass_utils import run_bass_kernel_spmd

F32 = mybir.dt.float32
BF16 = mybir.dt.bfloat16
AF = mybir.ActivationFunctionType
ALU = mybir.AluOpType

NEG = -1.0e30

# Full-problem constants
EMB = 2048
N_HEADS = 16
HEAD_DIM = 128
B_FULL = 4
S_FULL = 2048
N_CORES = 8
HPC = N_HEADS // N_CORES  # heads per core = 2


def build(B=B_FULL, S=S_FULL, E=EMB, hpc=HPC, DH=HEAD_DIM, CH=512):
    """Build the per-core Bass program (same program on all 8 cores)."""
    assert hpc == 2, "projection accumulation is written for 2 heads per core"
    SB = B * S
    DHC = hpc * DH          # per-core head dims (256)
    NE = E // 128           # e-tiles (contraction tiles)
    NEH = NE // 2
    NCH = S // CH           # 512-wide chunks per sequence
    KPC = CH // 128         # k-tiles per chunk (4)
    NST = S // 128          # 128-row s-tiles per sequence
    NOC = E // CH           # output chunks
    scale = 1.0 / math.sqrt(DH)

    nc = bacc.Bacc("TRN2", target_bir_lowering=False, debug=False,
                   num_devices=N_CORES)

    xT = nc.dram_tensor("xT", [E, SB], BF16, kind="ExternalInput")
    wqT = nc.dram_tensor("wqT", [E, DHC], BF16, kind="ExternalInput")
    wkT = nc.dram_tensor("wkT", [E, DHC], BF16, kind="ExternalInput")
    wvT = nc.dram_tensor("wvT", [E, DHC], BF16, kind="ExternalInput")
    woT = nc.dram_tensor("woT", [DHC, E], BF16, kind="ExternalInput")
    masks = nc.dram_tensor("masks", [128, CH], BF16, kind="ExternalInput")
    ones = nc.dram_tensor("ones", [128, 1], BF16, kind="ExternalInput")
    y = nc.dram_tensor("y", [SB, E], BF16, kind="ExternalOutput")

    with tile.TileContext(nc) as tc:
        with (
            tc.tile_pool(name="wpool", bufs=1) as wpool,
            tc.tile_pool(name="xtp", bufs=2) as xtp,
            tc.tile_pool(name="qkv", bufs=1) as qkv,
            tc.tile_pool(name="expp", bufs=4) as expp,
            tc.tile_pool(name="accp", bufs=2) as accp,
            tc.tile_pool(name="denp", bufs=2) as denp_sb,
            tc.tile_pool(name="yout", bufs=3) as yout,
            tc.tile_pool(name="ps_mm", bufs=3, space="PSUM") as ps_mm,
            tc.tile_pool(name="ps_proj", bufs=2, space="PSUM") as ps_proj,
            tc.tile_pool(name="ps_av", bufs=2, space="PSUM") as ps_av,
            tc.tile_pool(name="ps_den", bufs=1, space="PSUM") as ps_den,
        ):
            # Resident weights / constants.  The first Q chain consumes
            # e-tiles in order, so stage the DMAs so its head-of-stream
            # tiles (wq + x quarter 0) land first.
            wq_sb = wpool.tile([128, NE, DHC], BF16, tag="wq")
            wk_sb = wpool.tile([128, NE, DHC], BF16, tag="wk")
            wv_sb = wpool.tile([128, NE, DHC], BF16, tag="wv")
            wo_sb = wpool.tile([128, hpc, E], BF16, tag="wo")
            xT_r = xT.rearrange("(t p) s -> p t s", p=128)
            wq_r = wqT.rearrange("(t p) d -> p t d", p=128)
            wk_r = wkT.rearrange("(t p) d -> p t d", p=128)

            x0a = xtp.tile([128, NEH, CH], BF16, tag="xta", name="x0a")
            x0b = xtp.tile([128, NEH, CH], BF16, tag="xtb", name="x0b")
            nc.sync.dma_start(wq_sb[:, 0:2, :], wq_r[:, 0:2, :])
            nc.sync.dma_start(x0a[:, 0:2, :], xT_r[:, 0:2, 0:CH])
            nc.sync.dma_start(wq_sb[:, 2:4, :], wq_r[:, 2:4, :])
            nc.sync.dma_start(x0a[:, 2:4, :], xT_r[:, 2:4, 0:CH])
            nc.sync.dma_start(wq_sb[:, 4:NEH, :], wq_r[:, 4:NEH, :])
            nc.sync.dma_start(x0a[:, 4:NEH, :], xT_r[:, 4:NEH, 0:CH])
            nc.sync.dma_start(wq_sb[:, NEH:12, :], wq_r[:, NEH:12, :])
            nc.sync.dma_start(x0b[:, 0:4, :], xT_r[:, NEH:12, 0:CH])
            nc.sync.dma_start(wq_sb[:, 12:NE, :], wq_r[:, 12:NE, :])
            nc.sync.dma_start(x0b[:, 4:NEH, :], xT_r[:, 12:NE, 0:CH])
            nc.sync.dma_start(wk_sb[:, 0:NEH, :], wk_r[:, 0:NEH, :])
            nc.sync.dma_start(wk_sb[:, NEH:NE, :], wk_r[:, NEH:NE, :])
            xpre0 = ((0, 0), x0a, x0b)
            nc.sync.dma_start(wv_sb[:], wvT.rearrange("(t p) d -> p t d", p=128))
            nc.sync.dma_start(wo_sb[:], woT.rearrange("(h p) e -> p h e", p=128))
            mask_sb = wpool.tile([128, CH], BF16, tag="mask")
            nc.sync.dma_start(mask_sb[:], masks[:, :])
            ones_sb = wpool.tile([128, 1], BF16, tag="ones")
            nc.sync.dma_start(ones_sb[:], ones[:, :])

            # ---- output-projection job queue --------------------------
            # A job is one y tile (st, oc): both heads' partials accumulate
            # into one PSUM tile, single-op eviction (alternating engine),
            # DMA from the Sync engine (Pool stays light).
            proj_jobs = []
            evict_flip = [0]
            pending_den = [None]

            def drain_den():
                if pending_den[0] is not None:
                    job, pending_den[0] = pending_den[0], None
                    job()

            def emit_proj_job(job):
                ctxn, s0, st, oc = job
                o0 = oc * CH
                p = ps_proj.tile([128, CH], F32, tag="proj")
                nc.tensor.matmul(p[:], ctxn[:, 0, st * 128:(st + 1) * 128],
                                 wo_sb[:, 0, o0:o0 + CH],
                                 start=True, stop=False)
                nc.tensor.matmul(p[:], ctxn[:, 1, st * 128:(st + 1) * 128],
                                 wo_sb[:, 1, o0:o0 + CH],
                                 start=False, stop=True)
                ysb = yout.tile([128, CH], BF16, tag="ysb")
                if evict_flip[0] == 0:
                    nc.scalar.copy(ysb[:], p[:])
                else:
                    nc.vector.tensor_copy(ysb[:], p[:])
                evict_flip[0] ^= 1
                nc.sync.dma_start(
                    y[s0 + st * 128:s0 + (st + 1) * 128, o0:o0 + CH], ysb[:])

            def pop_proj(n=1):
                for _ in range(min(n, len(proj_jobs))):
                    emit_proj_job(proj_jobs.pop(0))

            xpre = xpre0
            for b in range(B):
                s0 = b * S
                # ---------------- Phase A: Q/K/V projections -------------
                qT = qkv.tile([128, hpc, S], BF16, tag="qT")
                kT = qkv.tile([128, hpc, S], BF16, tag="kT")
                v_sb = qkv.tile([128, NST, DHC], BF16, tag="v")
                for ch in range(NCH):
                    c0 = ch * CH
                    if xpre is not None and xpre[0] == (b, ch):
                        xta, xtb = xpre[1], xpre[2]
                    else:
                        xta = xtp.tile([128, NEH, CH], BF16, tag="xta")
                        nc.sync.dma_start(xta[:],
                                          xT_r[:, 0:NEH, s0 + c0:s0 + c0 + CH])
                        xtb = xtp.tile([128, NEH, CH], BF16, tag="xtb")
                        nc.sync.dma_start(xtb[:],
                                          xT_r[:, NEH:NE, s0 + c0:s0 + c0 + CH])
                    if ch + 1 < NCH or b + 1 < B:
                        nb_, nch = (b, ch + 1) if ch + 1 < NCH else (b + 1, 0)
                        n0 = nb_ * S + nch * CH
                        xna = xtp.tile([128, NEH, CH], BF16, tag="xta",
                                       name="xna")
                        nc.sync.dma_start(xna[:], xT_r[:, 0:NEH, n0:n0 + CH])
                        xnb = xtp.tile([128, NEH, CH], BF16, tag="xtb",
                                       name="xnb")
                        nc.sync.dma_start(xnb[:], xT_r[:, NEH:NE, n0:n0 + CH])
                        xpre = ((nb_, nch), xna, xnb)
                    else:
                        xpre = None

                    def xslice(et, lo=None, hi=None):
                        t = xta if et < NEH else xtb
                        e = et if et < NEH else et - NEH
                        if lo is None:
                            return t[:, e, :]
                        return t[:, e, lo:hi]

                    for h in range(hpc):
                        qp = ps_mm.tile([128, CH], F32, tag="qkvp")
                        for et in range(NE):
                            nc.tensor.matmul(
                                qp[:], wq_sb[:, et, h * DH:(h + 1) * DH],
                                xslice(et),
                                start=(et == 0), stop=(et == NE - 1))
                        nc.scalar.activation(qT[:, h, c0:c0 + CH], qp[:],
                                             AF.Identity, scale=scale)
                        drain_den()
                        pop_proj()
                        kp = ps_mm.tile([128, CH], F32, tag="qkvp")
                        for et in range(NE):
                            nc.tensor.matmul(
                                kp[:], wk_sb[:, et, h * DH:(h + 1) * DH],
                                xslice(et),
                                start=(et == 0), stop=(et == NE - 1))
                        nc.scalar.activation(kT[:, h, c0:c0 + CH], kp[:],
                                             AF.Identity)
                        pop_proj()
                    for st in range(KPC):
                        vp = ps_mm.tile([128, DHC], F32, tag="qkvp")
                        for et in range(NE):
                            nc.tensor.matmul(
                                vp[:], xslice(et, st * 128, (st + 1) * 128),
                                wv_sb[:, et, :],
                                start=(et == 0), stop=(et == NE - 1))
                        nc.scalar.activation(v_sb[:, ch * KPC + st, :], vp[:],
                                             AF.Identity)
                        pop_proj()

                # Flush leftover proj jobs of the previous batch down to 3:
                # the first attention unit's three pop slots drain those
                # before this batch's first ctx eviction is emitted (so the
                # recycled ctxn buffer is never read late), and they fill
                # the PE during the early units whose own jobs don't exist
                # yet.
                pop_proj(max(0, len(proj_jobs) - 3))

                # ------- Phase B: attention, software-pipelined ----------
                # Per (chunk g, head h): scores (transposed) and attn@V on
                # PE; the scores matmul for k-tile t+1 issues before attn@V
                # of t so Exp hides.  Exp tiles also accumulate on Vector
                # (bf16) into acc; the denominator chain (ones-by-acc
                # matmul, approx reciprocal, partition broadcast, normalized
                # ctx eviction) is deferred into the NEXT unit's k-tile loop
                # so none of its latency blocks the PE.  Proj jobs drain one
                # per k-tile step as PE filler.
                ctxn = qkv.tile([128, hpc, S], BF16, tag="ctxn")
                for g in range(NCH):
                    for h in range(hpc):
                        nk = KPC * (g + 1)
                        avp = ps_av.tile([128, CH], F32, tag="av")
                        acc = accp.tile([128, CH], BF16, tag="acc")
                        exs = [None] * nk
                        offs = [0] * nk

                        def emit_sp_exp(kt):
                            j = kt - (nk - KPC)
                            off = 128 * j if j > 0 else 0
                            w = CH - off
                            offs[kt] = off
                            sp = ps_mm.tile([128, CH], F32, tag="qkvp",
                                            name="sp")
                            nc.tensor.matmul(
                                sp[:, off:], kT[:, h, kt * 128:(kt + 1) * 128],
                                qT[:, h, g * CH + off:(g + 1) * CH],
                                start=True, stop=True)
                            if j >= 0:
                                # the triangular mask only spans the first
                                # 128 columns of the suffix; beyond that the
                                # additive mask is all zeros
                                mw = min(w, 128)
                                nc.vector.tensor_add(
                                    sp[:, off:off + mw], sp[:, off:off + mw],
                                    mask_sb[:, 0:mw])
                            ex = expp.tile([128, CH], BF16, tag="ex")
                            nc.scalar.activation(ex[:, off:], sp[:, off:],
                                                 AF.Exp)
                            exs[kt] = ex

                        def emit_av_acc(kt):
                            off = offs[kt]
                            nc.tensor.matmul(
                                avp[:, off:],
                                v_sb[:, kt, h * DH:(h + 1) * DH],
                                exs[kt][:, off:],
                                start=(kt == 0), stop=(kt == nk - 1),
                                skip_group_check=True)
                            if kt == 0:
                                nc.vector.tensor_copy(acc[:], exs[kt][:])
                            else:
                                nc.vector.tensor_add(acc[:, off:],
                                                     acc[:, off:],
                                                     exs[kt][:, off:])
                            exs[kt] = None

                        # lookahead 2: two scores matmuls stay in flight
                        # ahead of attn@V, so the Exp queue lag at unit
                        # start never exposes on the PE
                        for kt in range(nk):
                            emit_sp_exp(kt)
                            if kt == 1:
                                drain_den()
                            if kt >= 2:
                                emit_av_acc(kt - 2)
                                pop_proj()
                        emit_av_acc(nk - 2)
                        pop_proj()
                        emit_av_acc(nk - 1)

                        def den_job(g=g, h=h, avp=avp, acc=acc, ctxn=ctxn,
                                    s0=s0):
                            dnp = ps_den.tile([1, CH], F32, tag="den")
                            nc.tensor.matmul(dnp[:], ones_sb[:], acc[:],
                                             start=True, stop=True)
                            rrow = denp_sb.tile([1, CH], F32, tag="rrow")
                            nc.vector.reciprocal_approx_fast(rrow[:], dnp[:])
                            rbc = denp_sb.tile([128, CH], F32, tag="rbc")
                            nc.gpsimd.partition_broadcast(rbc[:], rrow[:])
                            nc.vector.tensor_tensor(
                                ctxn[:, h, g * CH:(g + 1) * CH], avp[:],
                                rbc[:], op=ALU.mult)
                            if h == 1:
                                for st in range(g * KPC, (g + 1) * KPC):
                                    for oc in range(NOC):
                                        proj_jobs.append((ctxn, s0, st, oc))
                        pending_den[0] = den_job
            drain_den()
            pop_proj(len(proj_jobs))
    nc.finalize()
    return nc


def host_consts(S=S_FULL, CH=512):
    """Mask / ones constant inputs."""
    p = np.arange(128)[:, None]
    c = np.arange(CH)[None, :]
    # strict lower triangle: masked iff c < p (diagonal k-tile suffix mask)
    masks = np.where(c < p, np.float32(NEG), np.float32(0.0))
    masks = np.ascontiguousarray(masks.astype(ml_dtypes.bfloat16))
    return {
        "masks": masks,
        "ones": np.ones((128, 1), dtype=ml_dtypes.bfloat16),
    }


def host_inputs(x, Wq, Wk, Wv, Wo, B=B_FULL, S=S_FULL, E=EMB, hpc=HPC,
                DH=HEAD_DIM, CH=512):
    """Shard + lay out the full inputs for the 8 cores (bf16 on device)."""
    SB = B * S
    DHC = hpc * DH
    bf = ml_dtypes.bfloat16
    xT = np.ascontiguousarray(x.reshape(SB, E).T.astype(bf))
    consts = host_consts(S, CH)

    in_maps = []
    for c in range(N_CORES):
        lo, hi = c * DHC, (c + 1) * DHC
        in_maps.append({
            "xT": xT,
            "wqT": np.ascontiguousarray(Wq[lo:hi, :].T.astype(bf)),
            "wkT": np.ascontiguousarray(Wk[lo:hi, :].T.astype(bf)),
            "wvT": np.ascontiguousarray(Wv[lo:hi, :].T.astype(bf)),
            "woT": np.ascontiguousarray(Wo[:, lo:hi].T.astype(bf)),
            **consts,
        })
    return in_maps


def kernel(x, Wq, Wk, Wv, Wo, bo):
    x = np.asarray(x, dtype=np.float32)
    Wq = np.asarray(Wq, dtype=np.float32)
    Wk = np.asarray(Wk, dtype=np.float32)
    Wv = np.asarray(Wv, dtype=np.float32)
    Wo = np.asarray(Wo, dtype=np.float32)
    bo = np.asarray(bo, dtype=np.float32)

    nc = build()
    in_maps = host_inputs(x, Wq, Wk, Wv, Wo)
    res = run_bass_kernel_spmd(nc, in_maps, list(range(N_CORES)))
    y = res.results[0]["y"].astype(np.float32)
    for c in range(1, N_CORES):
        y += res.results[c]["y"].astype(np.float32)
    y = (y + bo).astype(np.float32)
    return y.reshape(B_FULL, S_FULL, EMB)
